# revision 7
# baseline (speedup 1.0000x reference)
"""Trainium2 Bass kernel for CrossAttention.

  out = softmax(cos_sim(l2n(Q@WQ^T), l2n(K@WK^T)) * D^-0.5) @ l2n(V@WV^T) + Q

Shapes (full): query [16,2048,512] f32, key/value [16,2048,256] f32,
WQ [256,512], WK [256,256], WV [512,256].  Output [16,2048,512] f32.

Sharding: data-parallel over batch B=16 across 8 NeuronCores (2 batches per
core), no collectives.  One SPMD Bass program; per-core inputs differ.

Per-core per-batch dataflow (all matmuls bf16 operands, fp32 PSUM accum):
  1. Host pre-transposes Q/K/V to [feat, n] bf16 so the contraction dim (feat)
     lands on SBUF partitions; weights host-pre-transposed to [in, out] bf16.
  2. Projections run with the input tiles as the stationary operand, giving
     natural-layout w_q [n,256] / w_k [n,256] / w_v [n,512] tiles in PSUM.
  3. l2norm: copy PSUM->SBUF bf16 (unnormalized), DVE square-reduce per row,
     batched inv_norm = exp(-0.5*ln(sumsq)) on ACT (same table set as the
     softmax exp -> a single ACT table load for the whole kernel), then
     normalize in place.
  4. w_q/w_k tiles are PE-transposed to w_qT/w_kT [256, n] so the QK^T matmul
     directly produces dotT[k, q] tiles [128k, 512q].
  5. expT = exp(dotT * D^-0.5) -> SBUF bf16.  No max subtraction needed:
     dot entries are cosines in [-1,1], scaled by 1/16.
  6. out[q,v] = (expT.T @ w_v), rowsum[q] = (expT.T @ ones) accumulated over
     k tiles in PSUM; final = out * (1/rowsum) + Q_fp32 (residual), DMA out.
"""

import os
import time

import numpy as np
import ml_dtypes

import concourse.bass as bass
import concourse.bacc as bacc
import concourse.mybir as mybir
import concourse.tile as tile
from concourse.masks import make_identity

N_CORES = 8
P = 128
F = 512    # query feature dim
FK = 256   # key/value feature dim
D = 256    # qk projection dim
V = 512    # value projection dim (== output feature dim)

BF16 = mybir.dt.bfloat16
F32 = mybir.dt.float32
MULT = mybir.AluOpType.mult
ADD = mybir.AluOpType.add
EXP = mybir.ActivationFunctionType.Exp
LN = mybir.ActivationFunctionType.Ln


def build_core_program(bpc=2, nq=2048, nk=2048, stage=4):
    """Builds the single-core Bass program processing `bpc` batches.

    stage: debug bisection — 1 = projections+norm only, 2 = +transposes,
    3 = +dot/exp, 4 = full kernel.
    """
    nc = bacc.Bacc(
        "TRN2", target_bir_lowering=False, debug=False, num_devices=N_CORES
    )
    FT, FKT, DT = F // P, FK // P, D // P
    NQT, NKT = nq // P, nk // P
    QCH = 512                  # q-column chunk for the dot/exp stage
    NCH = nq // QCH
    SUB = QCH // P
    TEMP = float(D) ** -0.5

    qt_d = nc.dram_tensor("qt_in", [bpc, F, nq], BF16, kind="ExternalInput")
    kt_d = nc.dram_tensor("kt_in", [bpc, FK, nk], BF16, kind="ExternalInput")
    vt_d = nc.dram_tensor("vt_in", [bpc, FK, nk], BF16, kind="ExternalInput")
    qres_d = nc.dram_tensor("qres_in", [bpc, nq, F], F32, kind="ExternalInput")
    wqt_d = nc.dram_tensor("wqt_in", [F, D], BF16, kind="ExternalInput")
    wkt_d = nc.dram_tensor("wkt_in", [FK, D], BF16, kind="ExternalInput")
    wvt_d = nc.dram_tensor("wvt_in", [FK, V], BF16, kind="ExternalInput")
    out_d = nc.dram_tensor("out", [bpc, nq, F], F32, kind="ExternalOutput")

    with tile.TileContext(nc) as tc:
        with (
            tc.tile_pool(name="consts", bufs=1) as consts,
            tc.tile_pool(name="io", bufs=2) as io,
            tc.tile_pool(name="proj", bufs=2) as proj,
            tc.tile_pool(name="attn", bufs=2) as attn,
            tc.tile_pool(name="ps", bufs=2, space="PSUM") as ps,
        ):
            # --- constants ---
            wqt_sb = []
            for f in range(FT):
                w = consts.tile([P, D], BF16, name=f"wqt_sb{f}", tag=f"wqt{f}")
                nc.sync.dma_start(out=w, in_=wqt_d[f * P:(f + 1) * P, :])
                wqt_sb.append(w)
            wkt_sb = []
            for f in range(FKT):
                w = consts.tile([P, D], BF16, name=f"wkt_sb{f}", tag=f"wkt{f}")
                nc.sync.dma_start(out=w, in_=wkt_d[f * P:(f + 1) * P, :])
                wkt_sb.append(w)
            wvt_sb = []
            for f in range(FKT):
                w = consts.tile([P, V], BF16, name=f"wvt_sb{f}", tag=f"wvt{f}")
                nc.sync.dma_start(out=w, in_=wvt_d[f * P:(f + 1) * P, :])
                wvt_sb.append(w)
            ident = consts.tile([P, P], BF16, name="ident")
            make_identity(nc, ident)
            ones_sb = consts.tile([P, 1], BF16, name="ones_sb")
            nc.vector.memset(ones_sb, 1.0)

            def project_normalize(b, name, in_tiles, nt, w_sb_list, dim, w_tiles,
                                  wT_sb=None):
                """GEMM-project `in_tiles` (stationary) against weight tiles,
                l2-normalize rows, leaving normalized bf16 tiles in w_tiles.
                If wT_sb given ([DT][P, n] tiles), also PE-transpose into it."""
                nft = len(in_tiles)
                sumsq = proj.tile([P, nt], F32, name=f"sumsq_{name}_{b}",
                                  tag=f"sumsq_{name}", bufs=2)
                for n in range(nt):
                    pp = ps.tile([P, V], F32, name=f"ps_{name}_{b}_{n}",
                                 tag="proj")
                    pslice = pp[:, 0:dim]
                    for f in range(nft):
                        nc.tensor.matmul(
                            pslice,
                            lhsT=in_tiles[f][:, n * P:(n + 1) * P],
                            rhs=w_sb_list[f],
                            start=(f == 0),
                            stop=(f == nft - 1),
                        )
                    wt = proj.tile([P, dim], BF16, name=f"w_{name}_{b}_{n}",
                                   tag=f"w_{name}", bufs=nt + 4)
                    nc.vector.tensor_copy(out=wt, in_=pslice)
                    # row sum-of-squares via ACT Square+accum (same ACT table
                    # set as Ln/Exp -> one table load for the whole kernel)
                    sq = proj.tile([P, V], F32, name=f"sq_{name}_{b}_{n}",
                                   tag="sqscratch", bufs=2)
                    nc.scalar.activation(
                        out=sq[:, 0:dim], in_=pslice,
                        func=mybir.ActivationFunctionType.Square,
                        accum_out=sumsq[:, n:n + 1],
                    )
                    w_tiles.append(wt)
                # inv_norm = sumsq^-0.5 for all tiles at once (ln+exp: same ACT
                # table set as the softmax exp)
                lntmp = proj.tile([P, nt], F32, name=f"lntmp_{name}_{b}",
                                  tag=f"lntmp_{name}", bufs=2)
                nc.scalar.activation(out=lntmp, in_=sumsq, func=LN)
                inv = proj.tile([P, nt], F32, name=f"inv_{name}_{b}",
                                tag=f"inv_{name}", bufs=2)
                nc.scalar.activation(out=inv, in_=lntmp, func=EXP, scale=-0.5)
                for n in range(nt):
                    nc.vector.tensor_scalar_mul(
                        out=w_tiles[n], in0=w_tiles[n], scalar1=inv[:, n:n + 1]
                    )
                if wT_sb is not None:
                    for n in range(nt):
                        for d2 in range(dim // P):
                            pt = ps.tile([P, P], BF16,
                                         name=f"pst_{name}_{b}_{n}_{d2}",
                                         tag="proj")
                            nc.tensor.transpose(
                                pt, in_=w_tiles[n][:, d2 * P:(d2 + 1) * P],
                                identity=ident,
                            )
                            nc.vector.tensor_copy(
                                out=wT_sb[d2][:, n * P:(n + 1) * P], in_=pt
                            )

            for b in range(bpc):
                # --- load transposed inputs ---
                qt_sb = []
                for f in range(FT):
                    t = io.tile([P, nq], BF16, name=f"qt_sb_{b}_{f}", tag="qt",
                                bufs=FT + 2)
                    nc.sync.dma_start(out=t, in_=qt_d[b, f * P:(f + 1) * P, :])
                    qt_sb.append(t)
                kt_sb = []
                for f in range(FKT):
                    t = io.tile([P, nk], BF16, name=f"kt_sb_{b}_{f}", tag="kt",
                                bufs=FKT + 1)
                    nc.sync.dma_start(out=t, in_=kt_d[b, f * P:(f + 1) * P, :])
                    kt_sb.append(t)
                vt_sb = []
                for f in range(FKT):
                    t = io.tile([P, nk], BF16, name=f"vt_sb_{b}_{f}", tag="vt",
                                bufs=FKT + 1)
                    nc.sync.dma_start(out=t, in_=vt_d[b, f * P:(f + 1) * P, :])
                    vt_sb.append(t)

                # --- projections + l2norm (+ transpose for q/k) ---
                wqT_sb = [
                    attn.tile([P, nq], BF16, name=f"wqT_sb_{b}_{d}",
                              tag=f"wqT{d}", bufs=2)
                    for d in range(DT)
                ]
                wkT_sb = [
                    attn.tile([P, nk], BF16, name=f"wkT_sb_{b}_{d}",
                              tag=f"wkT{d}", bufs=2)
                    for d in range(DT)
                ]
                wq_tiles, wk_tiles, wv_tiles = [], [], []
                do_transpose = stage >= 2
                with nc.named_scope(f"proj_q_b{b}"):
                    project_normalize(b, "q", qt_sb, NQT, wqt_sb, D, wq_tiles,
                                      wT_sb=wqT_sb if do_transpose else None)
                with nc.named_scope(f"proj_k_b{b}"):
                    project_normalize(b, "k", kt_sb, NKT, wkt_sb, D, wk_tiles,
                                      wT_sb=wkT_sb if do_transpose else None)
                with nc.named_scope(f"proj_v_b{b}"):
                    project_normalize(b, "v", vt_sb, NKT, wvt_sb, V, wv_tiles)

                if stage <= 2:
                    # debug: dump normalized w_v tiles (f32-cast) to out rows
                    for n in range(NKT):
                        dbg = attn.tile([P, V], F32, name=f"dbg_{b}_{n}",
                                        tag="out_sb", bufs=4)
                        nc.vector.tensor_copy(out=dbg, in_=wv_tiles[n])
                        nc.sync.dma_start(
                            out=out_d[b, n * P:(n + 1) * P, :], in_=dbg)
                    continue

                # --- attention ---
                with nc.named_scope(f"attn_b{b}"):
                    for ch in range(NCH):
                        qs = slice(ch * QCH, (ch + 1) * QCH)
                        expt = []
                        for k in range(NKT):
                            dps = ps.tile([P, QCH], F32,
                                          name=f"dot_{b}_{ch}_{k}", tag="dot")
                            for d in range(DT):
                                nc.tensor.matmul(
                                    dps,
                                    lhsT=wkT_sb[d][:, k * P:(k + 1) * P],
                                    rhs=wqT_sb[d][:, qs],
                                    start=(d == 0),
                                    stop=(d == DT - 1),
                                )
                            et = attn.tile([P, QCH], BF16,
                                           name=f"expt_{b}_{ch}_{k}",
                                           tag="expt", bufs=NKT + 8)
                            nc.scalar.activation(out=et, in_=dps, func=EXP,
                                                 scale=TEMP)
                            expt.append(et)
                        if stage <= 3:
                            # debug: dump the exp tiles of the first chunk
                            for s in range(SUB):
                                rows = slice((ch * SUB + s) * P,
                                             (ch * SUB + s + 1) * P)
                                dbg = attn.tile([P, QCH], F32,
                                                name=f"dbge_{b}_{ch}_{s}",
                                                tag="out_sb", bufs=4)
                                nc.vector.tensor_copy(
                                    out=dbg, in_=expt[s * NKT // SUB])
                                nc.sync.dma_start(out=out_d[b, rows, 0:QCH],
                                                  in_=dbg)
                            continue
                        for s in range(SUB):
                            qt_idx = ch * SUB + s
                            rows = slice(qt_idx * P, (qt_idx + 1) * P)
                            qres_t = attn.tile([P, F], F32,
                                               name=f"qres_{b}_{qt_idx}",
                                               tag="qres", bufs=6)
                            nc.sync.dma_start(out=qres_t,
                                              in_=qres_d[b, rows, :])
                            ops_t = ps.tile([P, V], F32,
                                            name=f"outps_{b}_{qt_idx}",
                                            tag="out")
                            sps_t = ps.tile([P, 1], F32,
                                            name=f"sumps_{b}_{qt_idx}",
                                            tag="sums")
                            for k in range(NKT):
                                lhs = expt[k][:, s * P:(s + 1) * P]
                                nc.tensor.matmul(ops_t, lhsT=lhs,
                                                 rhs=wv_tiles[k],
                                                 start=(k == 0),
                                                 stop=(k == NKT - 1))
                                nc.tensor.matmul(sps_t, lhsT=lhs, rhs=ones_sb,
                                                 start=(k == 0),
                                                 stop=(k == NKT - 1))
                            invs = attn.tile([P, 1], F32,
                                             name=f"invs_{b}_{qt_idx}",
                                             tag="invs", bufs=4)
                            nc.vector.reciprocal(out=invs, in_=sps_t)
                            out_sb = attn.tile([P, F], F32,
                                               name=f"out_sb_{b}_{qt_idx}",
                                               tag="out_sb", bufs=4)
                            nc.vector.scalar_tensor_tensor(
                                out=out_sb, in0=ops_t, scalar=invs, in1=qres_t,
                                op0=MULT, op1=ADD,
                            )
                            nc.sync.dma_start(out=out_d[b, rows, :],
                                              in_=out_sb)

    nc.compile()
    return nc


_CACHE = {}


def _get_program(bpc, nq, nk):
    key = (bpc, nq, nk)
    if key not in _CACHE:
        _CACHE[key] = build_core_program(bpc, nq, nk)
    return _CACHE[key]


def make_in_maps(query, key, value, WQ, WK, WV, n_cores=N_CORES):
    """Host-side shard + layout prep: bf16 casts and transposes."""
    bf = ml_dtypes.bfloat16
    B = query.shape[0]
    qt = np.ascontiguousarray(query.astype(bf).transpose(0, 2, 1))
    kt = np.ascontiguousarray(key.astype(bf).transpose(0, 2, 1))
    vt = np.ascontiguousarray(value.astype(bf).transpose(0, 2, 1))
    qres = np.ascontiguousarray(query.astype(np.float32))
    wqt = np.ascontiguousarray(WQ.astype(bf).T)
    wkt = np.ascontiguousarray(WK.astype(bf).T)
    wvt = np.ascontiguousarray(WV.astype(bf).T)
    bpc = B // n_cores
    in_maps = []
    for c in range(n_cores):
        sl = slice(c * bpc, (c + 1) * bpc)
        in_maps.append({
            "qt_in": qt[sl], "kt_in": kt[sl], "vt_in": vt[sl],
            "qres_in": qres[sl],
            "wqt_in": wqt, "wkt_in": wkt, "wvt_in": wvt,
        })
    return in_maps, bpc


class _Runner:
    """Owns the jitted PJRT executable for the SPMD bass program so repeat
    kernel() calls reuse the compiled NEFF and device-resident inputs can be
    timed without per-call host transfers."""

    def __init__(self, nc):
        import jax
        import concourse.mybir as _mybir
        from jax.experimental.shard_map import shard_map
        from jax.sharding import Mesh, PartitionSpec
        from concourse import bass2jax

        bass2jax.install_neuronx_cc_hook()
        self.jax = jax
        self.nc = nc
        partition_name = (
            nc.partition_id_tensor.name if nc.partition_id_tensor else None
        )
        in_names, out_names, out_avals, zero_outs = [], [], [], []
        for alloc in nc.m.functions[0].allocations:
            if not isinstance(alloc, _mybir.MemoryLocationSet):
                continue
            name = alloc.memorylocations[0].name
            if alloc.kind == "ExternalInput":
                if name != partition_name:
                    in_names.append(name)
            elif alloc.kind == "ExternalOutput":
                shape = tuple(alloc.tensor_shape)
                dtype = _mybir.dt.np(alloc.dtype)
                out_names.append(name)
                out_avals.append(jax.core.ShapedArray(shape, dtype))
                zero_outs.append(np.zeros(shape, dtype))
        self.in_names = in_names
        self.out_names = out_names
        self.out_avals = out_avals
        self.zero_outs = zero_outs
        n_params = len(in_names)
        n_outs = len(out_avals)
        all_in_names = list(in_names) + list(out_names)
        if partition_name is not None:
            all_in_names.append(partition_name)

        def _body(*args):
            operands = list(args)
            if partition_name is not None:
                operands.append(bass2jax.partition_id_tensor())
            outs = bass2jax._bass_exec_p.bind(
                *operands,
                out_avals=tuple(out_avals),
                in_names=tuple(all_in_names),
                out_names=tuple(out_names),
                lowering_input_output_aliases=(),
                sim_require_finite=True,
                sim_require_nnan=True,
                nc=nc,
            )
            return tuple(outs)

        devices = jax.devices()[:N_CORES]
        assert len(devices) == N_CORES, f"need {N_CORES} cores, {devices}"
        self.mesh = Mesh(np.asarray(devices), ("core",))
        in_specs = (PartitionSpec("core"),) * (n_params + n_outs)
        out_specs = (PartitionSpec("core"),) * n_outs
        self.sharded = jax.jit(
            shard_map(_body, mesh=self.mesh, in_specs=in_specs,
                      out_specs=out_specs, check_rep=False),
            donate_argnums=tuple(range(n_params, n_params + n_outs)),
            keep_unused=True,
        )

    def put_inputs(self, in_maps):
        from jax.sharding import NamedSharding, PartitionSpec
        sh = NamedSharding(self.mesh, PartitionSpec("core"))
        concat = [
            np.concatenate([np.asarray(m[name]) for m in in_maps], axis=0)
            for name in self.in_names
        ]
        return [self.jax.device_put(a, sh) for a in concat]

    def put_zeros(self):
        from jax.sharding import NamedSharding, PartitionSpec
        sh = NamedSharding(self.mesh, PartitionSpec("core"))
        return [
            self.jax.device_put(
                np.zeros((N_CORES * z.shape[0], *z.shape[1:]), z.dtype), sh
            )
            for z in self.zero_outs
        ]

    def run(self, in_dev):
        outs = self.sharded(*in_dev, *self.put_zeros())
        return [np.asarray(o) for o in outs]

    def timed_run(self, in_dev, n_iters=5):
        """Warm wall-clock timing with device-resident inputs. Returns
        (outs_np, best_seconds)."""
        best = float("inf")
        outs = None
        for _ in range(n_iters):
            zeros = self.put_zeros()
            for z in zeros:
                z.block_until_ready()
            t0 = time.perf_counter()
            outs = self.sharded(*in_dev, *zeros)
            for o in outs:
                o.block_until_ready()
            t1 = time.perf_counter()
            best = min(best, t1 - t0)
        return [np.asarray(o) for o in outs], best


_RUNNERS = {}


def _get_runner(bpc, nq, nk):
    key = (bpc, nq, nk)
    if key not in _RUNNERS:
        _RUNNERS[key] = _Runner(_get_program(bpc, nq, nk))
    return _RUNNERS[key]


LAST_TIME_S = None


def kernel(query, key, value, WQ, WK, WV):
    global LAST_TIME_S
    query = np.asarray(query)
    B, nq, _ = query.shape
    nk = np.asarray(key).shape[1]
    in_maps, bpc = make_in_maps(
        query, np.asarray(key), np.asarray(value),
        np.asarray(WQ), np.asarray(WK), np.asarray(WV),
    )
    runner = _get_runner(bpc, nq, nk)
    in_dev = runner.put_inputs(in_maps)
    if int(os.environ.get("KERNEL_TIME", "0")):
        outs, best = runner.timed_run(in_dev)
        LAST_TIME_S = best
        print(f"HW exec time: {int(best * 1e9)} ns")
    else:
        outs = runner.run(in_dev)
    # single output "out": global [N_CORES*bpc, nq, F]
    out = outs[0].reshape(B, nq, F)
    return out.astype(np.float32)


# revision 9
# speedup vs baseline: 20.3574x; 20.3574x over previous
"""Trainium2 Bass kernel for CrossAttention.

  out = softmax(cos_sim(l2n(Q@WQ^T), l2n(K@WK^T)) * D^-0.5) @ l2n(V@WV^T) + Q

Shapes (full): query [16,2048,512] f32, key/value [16,2048,256] f32,
WQ [256,512], WK [256,256], WV [512,256].  Output [16,2048,512] f32.

Sharding: data-parallel over batch B=16 across 8 NeuronCores (2 batches per
core), no collectives.  One SPMD Bass program; per-core inputs differ.

Per-core per-batch dataflow (all matmuls bf16 operands, fp32 PSUM accum):
  1. Host pre-transposes Q/K/V to [feat, n] bf16 so the contraction dim (feat)
     lands on SBUF partitions; weights host-pre-transposed to [in, out] bf16.
  2. Projections run with the input tiles as the stationary operand, giving
     natural-layout w_q [n,256] / w_k [n,256] / w_v [n,512] tiles in PSUM.
  3. l2norm: copy PSUM->SBUF bf16 (unnormalized), DVE square-reduce per row,
     batched inv_norm = exp(-0.5*ln(sumsq)) on ACT (same table set as the
     softmax exp -> a single ACT table load for the whole kernel), then
     normalize in place.
  4. w_q/w_k tiles are PE-transposed to w_qT/w_kT [256, n] so the QK^T matmul
     directly produces dotT[k, q] tiles [128k, 512q].
  5. expT = exp(dotT * D^-0.5) -> SBUF bf16.  No max subtraction needed:
     dot entries are cosines in [-1,1], scaled by 1/16.
  6. out[q,v] = (expT.T @ w_v), rowsum[q] = (expT.T @ ones) accumulated over
     k tiles in PSUM; final = out * (1/rowsum) + Q_fp32 (residual), DMA out.
"""

import os
import time

import numpy as np
import ml_dtypes

import concourse.bass as bass
import concourse.bacc as bacc
import concourse.mybir as mybir
import concourse.tile as tile
from concourse.masks import make_identity

N_CORES = 8
P = 128
F = 512    # query feature dim
FK = 256   # key/value feature dim
D = 256    # qk projection dim
V = 512    # value projection dim (== output feature dim)

BF16 = mybir.dt.bfloat16
F32 = mybir.dt.float32
MULT = mybir.AluOpType.mult
ADD = mybir.AluOpType.add
EXP = mybir.ActivationFunctionType.Exp
LN = mybir.ActivationFunctionType.Ln


def build_core_program(bpc=2, nq=2048, nk=2048, stage=4):
    """Builds the single-core Bass program processing `bpc` batches.

    stage: debug bisection — 1 = projections+norm only, 2 = +transposes,
    3 = +dot/exp, 4 = full kernel.
    """
    nc = bacc.Bacc(
        "TRN2", target_bir_lowering=False, debug=False, num_devices=N_CORES
    )
    FT, FKT, DT = F // P, FK // P, D // P
    NQT, NKT = nq // P, nk // P
    QCH = 512                  # q-column chunk for the dot/exp stage
    NCH = nq // QCH
    SUB = QCH // P
    TEMP = float(D) ** -0.5

    qt_d = nc.dram_tensor("qt_in", [bpc, F, nq], BF16, kind="ExternalInput")
    kt_d = nc.dram_tensor("kt_in", [bpc, FK, nk], BF16, kind="ExternalInput")
    vt_d = nc.dram_tensor("vt_in", [bpc, FK, nk], BF16, kind="ExternalInput")
    qres_d = nc.dram_tensor("qres_in", [bpc, nq, F], F32, kind="ExternalInput")
    wqt_d = nc.dram_tensor("wqt_in", [F, D], BF16, kind="ExternalInput")
    wkt_d = nc.dram_tensor("wkt_in", [FK, D], BF16, kind="ExternalInput")
    wvt_d = nc.dram_tensor("wvt_in", [FK, V], BF16, kind="ExternalInput")
    out_d = nc.dram_tensor("out", [bpc, nq, F], F32, kind="ExternalOutput")

    with tile.TileContext(nc) as tc:
        with (
            tc.tile_pool(name="consts", bufs=1) as consts,
            tc.tile_pool(name="io", bufs=2) as io,
            tc.tile_pool(name="proj", bufs=2) as proj,
            tc.tile_pool(name="attn", bufs=2) as attn,
            tc.tile_pool(name="ps", bufs=2, space="PSUM") as ps,
        ):
            # --- constants ---
            wqt_sb = []
            for f in range(FT):
                w = consts.tile([P, D], BF16, name=f"wqt_sb{f}", tag=f"wqt{f}")
                nc.sync.dma_start(out=w, in_=wqt_d[f * P:(f + 1) * P, :])
                wqt_sb.append(w)
            wkt_sb = []
            for f in range(FKT):
                w = consts.tile([P, D], BF16, name=f"wkt_sb{f}", tag=f"wkt{f}")
                nc.sync.dma_start(out=w, in_=wkt_d[f * P:(f + 1) * P, :])
                wkt_sb.append(w)
            wvt_sb = []
            for f in range(FKT):
                w = consts.tile([P, V], BF16, name=f"wvt_sb{f}", tag=f"wvt{f}")
                nc.sync.dma_start(out=w, in_=wvt_d[f * P:(f + 1) * P, :])
                wvt_sb.append(w)
            ident = consts.tile([P, P], BF16, name="ident")
            make_identity(nc, ident)
            ones_sb = consts.tile([P, 1], BF16, name="ones_sb")
            nc.vector.memset(ones_sb, 1.0)

            def project_normalize(b, name, in_tiles, nt, w_sb_list, dim, w_tiles,
                                  wT_sb=None):
                """GEMM-project `in_tiles` (stationary) against weight tiles,
                l2-normalize rows, leaving normalized bf16 tiles in w_tiles.
                If wT_sb given ([DT][P, n] tiles), also PE-transpose into it."""
                nft = len(in_tiles)
                sumsq = proj.tile([P, nt], F32, name=f"sumsq_{name}_{b}",
                                  tag=f"sumsq_{name}", bufs=2)
                for n in range(nt):
                    pp = ps.tile([P, V], F32, name=f"ps_{name}_{b}_{n}",
                                 tag="proj")
                    pslice = pp[:, 0:dim]
                    for f in range(nft):
                        nc.tensor.matmul(
                            pslice,
                            lhsT=in_tiles[f][:, n * P:(n + 1) * P],
                            rhs=w_sb_list[f],
                            start=(f == 0),
                            stop=(f == nft - 1),
                        )
                    wt = proj.tile([P, dim], BF16, name=f"w_{name}_{b}_{n}",
                                   tag=f"w_{name}", bufs=nt + 4)
                    nc.vector.tensor_copy(out=wt, in_=pslice)
                    # row sum-of-squares via ACT Square+accum (same ACT table
                    # set as Ln/Exp -> one table load for the whole kernel)
                    sq = proj.tile([P, V], F32, name=f"sq_{name}_{b}_{n}",
                                   tag="sqscratch", bufs=2)
                    nc.scalar.activation(
                        out=sq[:, 0:dim], in_=pslice,
                        func=mybir.ActivationFunctionType.Square,
                        accum_out=sumsq[:, n:n + 1],
                    )
                    w_tiles.append(wt)
                # inv_norm = sumsq^-0.5 for all tiles at once (ln+exp: same ACT
                # table set as the softmax exp)
                lntmp = proj.tile([P, nt], F32, name=f"lntmp_{name}_{b}",
                                  tag=f"lntmp_{name}", bufs=2)
                nc.scalar.activation(out=lntmp, in_=sumsq, func=LN)
                inv = proj.tile([P, nt], F32, name=f"inv_{name}_{b}",
                                tag=f"inv_{name}", bufs=2)
                nc.scalar.activation(out=inv, in_=lntmp, func=EXP, scale=-0.5)
                for n in range(nt):
                    nc.vector.tensor_scalar_mul(
                        out=w_tiles[n], in0=w_tiles[n], scalar1=inv[:, n:n + 1]
                    )
                if wT_sb is not None:
                    for n in range(nt):
                        for d2 in range(dim // P):
                            pt = ps.tile([P, P], BF16,
                                         name=f"pst_{name}_{b}_{n}_{d2}",
                                         tag="proj")
                            nc.tensor.transpose(
                                pt, in_=w_tiles[n][:, d2 * P:(d2 + 1) * P],
                                identity=ident,
                            )
                            nc.vector.tensor_copy(
                                out=wT_sb[d2][:, n * P:(n + 1) * P], in_=pt
                            )

            for b in range(bpc):
                # --- load transposed inputs ---
                qt_sb = []
                for f in range(FT):
                    t = io.tile([P, nq], BF16, name=f"qt_sb_{b}_{f}", tag="qt",
                                bufs=FT + 2)
                    nc.sync.dma_start(out=t, in_=qt_d[b, f * P:(f + 1) * P, :])
                    qt_sb.append(t)
                kt_sb = []
                for f in range(FKT):
                    t = io.tile([P, nk], BF16, name=f"kt_sb_{b}_{f}", tag="kt",
                                bufs=FKT + 1)
                    nc.sync.dma_start(out=t, in_=kt_d[b, f * P:(f + 1) * P, :])
                    kt_sb.append(t)
                vt_sb = []
                for f in range(FKT):
                    t = io.tile([P, nk], BF16, name=f"vt_sb_{b}_{f}", tag="vt",
                                bufs=FKT + 1)
                    nc.sync.dma_start(out=t, in_=vt_d[b, f * P:(f + 1) * P, :])
                    vt_sb.append(t)

                # --- projections + l2norm (+ transpose for q/k) ---
                wqT_sb = [
                    attn.tile([P, nq], BF16, name=f"wqT_sb_{b}_{d}",
                              tag=f"wqT{d}", bufs=2)
                    for d in range(DT)
                ]
                wkT_sb = [
                    attn.tile([P, nk], BF16, name=f"wkT_sb_{b}_{d}",
                              tag=f"wkT{d}", bufs=2)
                    for d in range(DT)
                ]
                wq_tiles, wk_tiles, wv_tiles = [], [], []
                do_transpose = stage >= 2
                with nc.named_scope(f"proj_q_b{b}"):
                    project_normalize(b, "q", qt_sb, NQT, wqt_sb, D, wq_tiles,
                                      wT_sb=wqT_sb if do_transpose else None)
                with nc.named_scope(f"proj_k_b{b}"):
                    project_normalize(b, "k", kt_sb, NKT, wkt_sb, D, wk_tiles,
                                      wT_sb=wkT_sb if do_transpose else None)
                with nc.named_scope(f"proj_v_b{b}"):
                    project_normalize(b, "v", vt_sb, NKT, wvt_sb, V, wv_tiles)

                if stage <= 2:
                    # debug: dump normalized w_v tiles (f32-cast) to out rows
                    for n in range(NKT):
                        dbg = attn.tile([P, V], F32, name=f"dbg_{b}_{n}",
                                        tag="out_sb", bufs=4)
                        nc.vector.tensor_copy(out=dbg, in_=wv_tiles[n])
                        nc.sync.dma_start(
                            out=out_d[b, n * P:(n + 1) * P, :], in_=dbg)
                    continue

                # --- attention ---
                with nc.named_scope(f"attn_b{b}"):
                    for ch in range(NCH):
                        qs = slice(ch * QCH, (ch + 1) * QCH)
                        expt = []
                        for k in range(NKT):
                            dps = ps.tile([P, QCH], F32,
                                          name=f"dot_{b}_{ch}_{k}", tag="dot")
                            for d in range(DT):
                                nc.tensor.matmul(
                                    dps,
                                    lhsT=wkT_sb[d][:, k * P:(k + 1) * P],
                                    rhs=wqT_sb[d][:, qs],
                                    start=(d == 0),
                                    stop=(d == DT - 1),
                                )
                            et = attn.tile([P, QCH], BF16,
                                           name=f"expt_{b}_{ch}_{k}",
                                           tag="expt", bufs=NKT + 8)
                            nc.scalar.activation(out=et, in_=dps, func=EXP,
                                                 scale=TEMP)
                            expt.append(et)
                        if stage <= 3:
                            # debug: dump the exp tiles of the first chunk
                            for s in range(SUB):
                                rows = slice((ch * SUB + s) * P,
                                             (ch * SUB + s + 1) * P)
                                dbg = attn.tile([P, QCH], F32,
                                                name=f"dbge_{b}_{ch}_{s}",
                                                tag="out_sb", bufs=4)
                                nc.vector.tensor_copy(
                                    out=dbg, in_=expt[s * NKT // SUB])
                                nc.sync.dma_start(out=out_d[b, rows, 0:QCH],
                                                  in_=dbg)
                            continue
                        for s in range(SUB):
                            qt_idx = ch * SUB + s
                            rows = slice(qt_idx * P, (qt_idx + 1) * P)
                            qres_t = attn.tile([P, F], F32,
                                               name=f"qres_{b}_{qt_idx}",
                                               tag="qres", bufs=6)
                            nc.sync.dma_start(out=qres_t,
                                              in_=qres_d[b, rows, :])
                            ops_t = ps.tile([P, V], F32,
                                            name=f"outps_{b}_{qt_idx}",
                                            tag="out")
                            sps_t = ps.tile([P, 1], F32,
                                            name=f"sumps_{b}_{qt_idx}",
                                            tag="sums")
                            for k in range(NKT):
                                lhs = expt[k][:, s * P:(s + 1) * P]
                                nc.tensor.matmul(ops_t, lhsT=lhs,
                                                 rhs=wv_tiles[k],
                                                 start=(k == 0),
                                                 stop=(k == NKT - 1))
                                nc.tensor.matmul(sps_t, lhsT=lhs, rhs=ones_sb,
                                                 start=(k == 0),
                                                 stop=(k == NKT - 1))
                            invs = attn.tile([P, 1], F32,
                                             name=f"invs_{b}_{qt_idx}",
                                             tag="invs", bufs=4)
                            nc.vector.reciprocal(out=invs, in_=sps_t)
                            out_sb = attn.tile([P, F], F32,
                                               name=f"out_sb_{b}_{qt_idx}",
                                               tag="out_sb", bufs=4)
                            nc.vector.scalar_tensor_tensor(
                                out=out_sb, in0=ops_t, scalar=invs, in1=qres_t,
                                op0=MULT, op1=ADD,
                            )
                            nc.sync.dma_start(out=out_d[b, rows, :],
                                              in_=out_sb)

    nc.compile()
    return nc


_CACHE = {}


def _get_program(bpc, nq, nk):
    key = (bpc, nq, nk)
    if key not in _CACHE:
        _CACHE[key] = build_core_program(bpc, nq, nk)
    return _CACHE[key]


def make_in_maps(query, key, value, WQ, WK, WV, n_cores=N_CORES):
    """Host-side shard + layout prep: bf16 casts and transposes."""
    bf = ml_dtypes.bfloat16
    B = query.shape[0]
    qt = np.ascontiguousarray(query.astype(bf).transpose(0, 2, 1))
    kt = np.ascontiguousarray(key.astype(bf).transpose(0, 2, 1))
    vt = np.ascontiguousarray(value.astype(bf).transpose(0, 2, 1))
    qres = np.ascontiguousarray(query.astype(np.float32))
    wqt = np.ascontiguousarray(WQ.astype(bf).T)
    wkt = np.ascontiguousarray(WK.astype(bf).T)
    wvt = np.ascontiguousarray(WV.astype(bf).T)
    bpc = B // n_cores
    in_maps = []
    for c in range(n_cores):
        sl = slice(c * bpc, (c + 1) * bpc)
        in_maps.append({
            "qt_in": qt[sl], "kt_in": kt[sl], "vt_in": vt[sl],
            "qres_in": qres[sl],
            "wqt_in": wqt, "wkt_in": wkt, "wvt_in": wvt,
        })
    return in_maps, bpc


class _Runner:
    """Owns the jitted PJRT executable for the SPMD bass program so repeat
    kernel() calls reuse the compiled NEFF and device-resident inputs can be
    timed without per-call host transfers."""

    def __init__(self, nc):
        import jax
        import concourse.mybir as _mybir
        from jax.experimental.shard_map import shard_map
        from jax.sharding import Mesh, PartitionSpec
        from concourse import bass2jax

        bass2jax.install_neuronx_cc_hook()
        self.jax = jax
        self.nc = nc
        partition_name = (
            nc.partition_id_tensor.name if nc.partition_id_tensor else None
        )
        in_names, out_names, out_avals, zero_outs = [], [], [], []
        for alloc in nc.m.functions[0].allocations:
            if not isinstance(alloc, _mybir.MemoryLocationSet):
                continue
            name = alloc.memorylocations[0].name
            if alloc.kind == "ExternalInput":
                if name != partition_name:
                    in_names.append(name)
            elif alloc.kind == "ExternalOutput":
                shape = tuple(alloc.tensor_shape)
                dtype = _mybir.dt.np(alloc.dtype)
                out_names.append(name)
                out_avals.append(jax.core.ShapedArray(shape, dtype))
                zero_outs.append(np.zeros(shape, dtype))
        self.in_names = in_names
        self.out_names = out_names
        self.out_avals = out_avals
        self.zero_outs = zero_outs
        n_params = len(in_names)
        n_outs = len(out_avals)
        all_in_names = list(in_names) + list(out_names)
        if partition_name is not None:
            all_in_names.append(partition_name)

        def _body(*args):
            operands = list(args)
            if partition_name is not None:
                operands.append(bass2jax.partition_id_tensor())
            outs = bass2jax._bass_exec_p.bind(
                *operands,
                out_avals=tuple(out_avals),
                in_names=tuple(all_in_names),
                out_names=tuple(out_names),
                lowering_input_output_aliases=(),
                sim_require_finite=True,
                sim_require_nnan=True,
                nc=nc,
            )
            return tuple(outs)

        devices = jax.devices()[:N_CORES]
        assert len(devices) == N_CORES, f"need {N_CORES} cores, {devices}"
        self.mesh = Mesh(np.asarray(devices), ("core",))
        in_specs = (PartitionSpec("core"),) * (n_params + n_outs)
        out_specs = (PartitionSpec("core"),) * n_outs
        self.sharded = jax.jit(
            shard_map(_body, mesh=self.mesh, in_specs=in_specs,
                      out_specs=out_specs, check_rep=False),
            donate_argnums=tuple(range(n_params, n_params + n_outs)),
            keep_unused=True,
        )

    def put_inputs(self, in_maps):
        from jax.sharding import NamedSharding, PartitionSpec
        sh = NamedSharding(self.mesh, PartitionSpec("core"))
        concat = [
            np.concatenate([np.asarray(m[name]) for m in in_maps], axis=0)
            for name in self.in_names
        ]
        return [self.jax.device_put(a, sh) for a in concat]

    def put_zeros(self):
        from jax.sharding import NamedSharding, PartitionSpec
        sh = NamedSharding(self.mesh, PartitionSpec("core"))
        return [
            self.jax.device_put(
                np.zeros((N_CORES * z.shape[0], *z.shape[1:]), z.dtype), sh
            )
            for z in self.zero_outs
        ]

    def run(self, in_dev):
        outs = self.sharded(*in_dev, *self.put_zeros())
        return [np.asarray(o) for o in outs]

    def timed_run(self, in_dev, n_iters=5):
        """Warm wall-clock timing with device-resident inputs. Returns
        (outs_np, best_seconds)."""
        best = float("inf")
        outs = None
        for _ in range(n_iters):
            zeros = self.put_zeros()
            for z in zeros:
                z.block_until_ready()
            t0 = time.perf_counter()
            outs = self.sharded(*in_dev, *zeros)
            for o in outs:
                o.block_until_ready()
            t1 = time.perf_counter()
            best = min(best, t1 - t0)
        return [np.asarray(o) for o in outs], best

    def measure_exec_ns(self, in_dev, k_lo=2, k_hi=16, n_reps=4):
        """Per-NEFF-execution time from the slope of python-chained runs:
        call the jitted executable k times back-to-back (iteration i's
        outputs are donated as iteration i+1's output buffers), blocking only
        at the end.  Async dispatch pipelines the executions on device, so
        wall(k_hi)-wall(k_lo) isolates per-execution device time from the
        axon-tunnel round-trip overhead."""

        def run_k(k):
            outs = tuple(self.put_zeros())
            for o in outs:
                o.block_until_ready()
            t0 = time.perf_counter()
            for _ in range(k):
                outs = self.sharded(*in_dev, *outs)
            for o in outs:
                o.block_until_ready()
            return time.perf_counter() - t0

        run_k(2)  # warmup
        lo = min(run_k(k_lo) for _ in range(n_reps))
        hi = min(run_k(k_hi) for _ in range(n_reps))
        per_exec = (hi - lo) / (k_hi - k_lo)
        return per_exec * 1e9, lo, hi


_RUNNERS = {}


def _get_runner(bpc, nq, nk):
    key = (bpc, nq, nk)
    if key not in _RUNNERS:
        _RUNNERS[key] = _Runner(_get_program(bpc, nq, nk))
    return _RUNNERS[key]


LAST_TIME_S = None


def kernel(query, key, value, WQ, WK, WV):
    global LAST_TIME_S
    query = np.asarray(query)
    B, nq, _ = query.shape
    nk = np.asarray(key).shape[1]
    in_maps, bpc = make_in_maps(
        query, np.asarray(key), np.asarray(value),
        np.asarray(WQ), np.asarray(WK), np.asarray(WV),
    )
    runner = _get_runner(bpc, nq, nk)
    in_dev = runner.put_inputs(in_maps)
    if int(os.environ.get("KERNEL_TIME", "0")):
        outs, best = runner.timed_run(in_dev)
        LAST_TIME_S = best
        print(f"HW exec time: {int(best * 1e9)} ns")
    else:
        outs = runner.run(in_dev)
    # single output "out": global [N_CORES*bpc, nq, F]
    out = outs[0].reshape(B, nq, F)
    return out.astype(np.float32)
